# revision 8
# baseline (speedup 1.0000x reference)
"""ArcMarginProduct (subcenter + inter-topk) Trainium2 kernel.

Math note: the reference uses mp=0.0, so phi_mp = cos*cos(0) + sine*sin(0)
== cos bitwise. The inter-topk term therefore cancels exactly:
    one_hot*phi + tk*phi_mp + (1-one_hot-tk)*cos == one_hot*phi + (1-one_hot)*cos
The kernel computes, per row r and class c:
    out[r, c] = 32 * max(cosine[r, 3c:3c+3])            for c != label[r]
    out[r, l] = 32 * phi(cos_l),  cos_l = max(cosine[r, 3l:3l+3])
    phi(x) = x*cos(m) - sqrt(max(1-x^2,0))*sin(m)  if x > cos(pi-m)
             x - (1 + cos(pi-m))                   otherwise

Sharding: batch dim across 8 NeuronCores (128 rows/core = SBUF partitions).
No collectives; the label fixup is local to each core's rows.

Engine notes: DVE ops with per-partition scalar pointer operands lower to
the S2S2D2_STT encoding which supports only one sync-wait — Tile may attach
more, so the hot loop uses only tensor_tensor (DVE) and activation (ACT)
instructions. The one-hot label mask is built on ACT:
    mask = Relu(1 - |iota + (j*W - label)|)
which is exact for integral values, then fix = Copy(mask * delta) with the
per-partition delta as the activation scale operand.
"""

import math
import sys

import numpy as np

if "/opt/trn_rl_repo" not in sys.path:
    sys.path.insert(0, "/opt/trn_rl_repo")

import concourse.bass as bass
import concourse.bacc as bacc
import concourse.mybir as mybir
from concourse.bass_utils import run_bass_kernel_spmd
from concourse.tile import TileContext

B = 1024
C = 20000          # out_features
K = 3              # subcenters
CK = C * K         # 60000
NCORES = 8
RB = B // NCORES   # 128 rows per core
W = 2000           # output columns per tile
NT = C // W        # tiles per core

SCALE = 32.0
MARGIN = 0.2
COS_M = math.cos(MARGIN)
SIN_M = math.sin(MARGIN)
TH = math.cos(math.pi - MARGIN)
MMM = 1.0 + math.cos(math.pi - MARGIN)

_CACHED_NC = None


def build():
    f32 = mybir.dt.float32
    i32 = mybir.dt.int32
    Alu = mybir.AluOpType
    Act = mybir.ActivationFunctionType

    nc = bacc.Bacc()
    cos_d = nc.declare_dram_parameter("cos", [RB, CK], f32, isOutput=False)
    gix_d = nc.declare_dram_parameter("gix", [RB, 1], i32, isOutput=False)
    neglab_d = nc.declare_dram_parameter("neglab", [RB, NT], f32, isOutput=False)
    out_d = nc.declare_dram_parameter("out", [RB, C], f32, isOutput=True)

    with TileContext(nc) as tc:
        with (
            tc.tile_pool(name="const", bufs=1) as cpool,
            tc.tile_pool(name="small", bufs=1) as spool,
            tc.tile_pool(name="inp", bufs=3) as ipool,
            tc.tile_pool(name="mid", bufs=2) as mpool,
        ):
            # iota 0..W-1 along the free dim, as f32 (exact for W < 2^24)
            iota_i = cpool.tile([RB, W], i32)
            nc.gpsimd.iota(iota_i[:], pattern=[[1, W]], base=0, channel_multiplier=0)
            iota_f = cpool.tile([RB, W], f32)
            nc.vector.tensor_copy(iota_f[:], iota_i[:])

            neglab_t = cpool.tile([RB, NT], f32)
            nc.sync.dma_start(out=neglab_t[:], in_=neglab_d[:])
            gix_t = cpool.tile([RB, 1], i32)
            nc.sync.dma_start(out=gix_t[:], in_=gix_d[:])

            # small per-partition constants (avoid const-AP registry)
            mmm_t = cpool.tile([RB, 1], f32)
            nc.gpsimd.memset(mmm_t[:], -MMM)
            th_t = cpool.tile([RB, 1], f32)
            nc.gpsimd.memset(th_t[:], TH)

            # Gather cosine[r, 3l : 3l+3] -> [RB, 3] via flat element offsets.
            g3 = spool.tile([RB, 3], f32)
            nc.gpsimd.indirect_dma_start(
                out=g3[:],
                out_offset=None,
                in_=cos_d[:].rearrange("p (n o) -> (p n) o", o=1),
                in_offset=bass.IndirectOffsetOnAxis(ap=gix_t[:, :1], axis=0),
            )
            cos_l = spool.tile([RB, 1], f32)
            nc.vector.tensor_reduce(
                out=cos_l[:], in_=g3[:], axis=mybir.AxisListType.X, op=Alu.max
            )

            # phi at the label column, then delta = (phi - cos_l) * SCALE.
            # All [RB,1] ops; only tensor_tensor / activation encodings.
            c2 = spool.tile([RB, 1], f32)
            nc.scalar.square(c2[:], cos_l[:])
            om = spool.tile([RB, 1], f32)
            nc.scalar.activation(om[:], c2[:], Act.Identity, bias=1.0, scale=-1.0)
            omc = spool.tile([RB, 1], f32)
            nc.scalar.activation(omc[:], om[:], Act.Relu)
            sine = spool.tile([RB, 1], f32)
            nc.scalar.sqrt(sine[:], omc[:])
            pa = spool.tile([RB, 1], f32)
            nc.scalar.mul(pa[:], cos_l[:], COS_M)
            pb = spool.tile([RB, 1], f32)
            nc.scalar.mul(pb[:], sine[:], -SIN_M)
            phi_b = spool.tile([RB, 1], f32)
            nc.vector.tensor_add(phi_b[:], pa[:], pb[:])
            cmm = spool.tile([RB, 1], f32)
            nc.vector.tensor_add(cmm[:], cos_l[:], mmm_t[:])
            mgt = spool.tile([RB, 1], f32)
            nc.vector.tensor_tensor(
                out=mgt[:], in0=cos_l[:], in1=th_t[:], op=Alu.is_gt
            )
            d1 = spool.tile([RB, 1], f32)
            nc.vector.tensor_sub(d1[:], phi_b[:], cmm[:])
            d2 = spool.tile([RB, 1], f32)
            nc.vector.tensor_mul(d2[:], d1[:], mgt[:])
            phi = spool.tile([RB, 1], f32)
            nc.vector.tensor_add(phi[:], d2[:], cmm[:])
            d3 = spool.tile([RB, 1], f32)
            nc.vector.tensor_sub(d3[:], phi[:], cos_l[:])
            delta = spool.tile([RB, 1], f32)
            nc.scalar.mul(delta[:], d3[:], SCALE)

            for j in range(NT):
                in3 = ipool.tile([RB, 3 * W], f32, tag="in3")
                nc.sync.dma_start(
                    out=in3[:], in_=cos_d[:, j * 3 * W : (j + 1) * 3 * W]
                )
                v = in3[:].rearrange("p (w k) -> p w k", k=3)
                t0 = mpool.tile([RB, W], f32, tag="t0")
                nc.vector.tensor_max(t0[:], v[:, :, 0], v[:, :, 1])
                cosr = mpool.tile([RB, W], f32, tag="cosr")
                nc.vector.tensor_max(cosr[:], t0[:], v[:, :, 2])
                cos32 = mpool.tile([RB, W], f32, tag="cos32")
                nc.scalar.mul(cos32[:], cosr[:], SCALE)

                # one-hot fix: mask = Relu(1 - |iota + (j*W - label)|)
                absd = mpool.tile([RB, W], f32, tag="absd")
                nc.scalar.activation(
                    absd[:], iota_f[:], Act.Abs,
                    bias=neglab_t[:, j : j + 1], scale=1.0,
                )
                mask = mpool.tile([RB, W], f32, tag="mask")
                nc.scalar.activation(
                    mask[:], absd[:], Act.Relu, bias=1.0, scale=-1.0
                )
                # fix = mask * delta  (delta as per-partition ACT scale)
                nc.scalar.activation(
                    absd[:], mask[:], Act.Copy, bias=0.0, scale=delta[:, :1]
                )
                nc.vector.tensor_add(cos32[:], cos32[:], absd[:])
                nc.sync.dma_start(out=out_d[:, j * W : (j + 1) * W], in_=cos32[:])

    nc.finalize()
    return nc


def _make_in_maps(cosine: np.ndarray, label: np.ndarray):
    in_maps = []
    rows = np.arange(RB, dtype=np.int64)
    jw = (np.arange(NT, dtype=np.int64) * W)[None, :]  # [1, NT]
    for i in range(NCORES):
        rs = slice(i * RB, (i + 1) * RB)
        lab = np.asarray(label[rs], dtype=np.int64)
        gix = (rows * CK + 3 * lab).astype(np.int32).reshape(RB, 1)
        neglab = (jw - lab[:, None]).astype(np.float32)  # [RB, NT]
        in_maps.append(
            {
                "cos": np.ascontiguousarray(cosine[rs], dtype=np.float32),
                "gix": gix,
                "neglab": np.ascontiguousarray(neglab),
            }
        )
    return in_maps


def kernel(cosine: np.ndarray, label: np.ndarray) -> np.ndarray:
    global _CACHED_NC
    cosine = np.asarray(cosine)
    label = np.asarray(label)
    assert cosine.shape == (B, CK), cosine.shape
    assert label.shape == (B,), label.shape

    if _CACHED_NC is None:
        _CACHED_NC = build()
    nc = _CACHED_NC

    in_maps = _make_in_maps(cosine, label)
    res = run_bass_kernel_spmd(nc, in_maps, core_ids=list(range(NCORES)))
    out = np.concatenate([res.results[i]["out"] for i in range(NCORES)], axis=0)
    return out.astype(np.float32, copy=False)


# revision 13
# speedup vs baseline: 1.1438x; 1.1438x over previous
"""ArcMarginProduct (subcenter + inter-topk) Trainium2 kernel.

Math note: the reference uses mp=0.0, so phi_mp = cos*cos(0) + sine*sin(0)
== cos bitwise. The inter-topk term therefore cancels exactly:
    one_hot*phi + tk*phi_mp + (1-one_hot-tk)*cos == one_hot*phi + (1-one_hot)*cos
The kernel computes, per row r and class c:
    out[r, c] = 32 * max(cosine[r, 3c:3c+3])            for c != label[r]
    out[r, l] = 32 * phi(cos_l),  cos_l = max(cosine[r, 3l:3l+3])
    phi(x) = x*cos(m) - sqrt(max(1-x^2,0))*sin(m)  if x > cos(pi-m)
             x - (1 + cos(pi-m))                   otherwise

Sharding: batch dim across 8 NeuronCores (128 rows/core = SBUF partitions).
No collectives; the label fixup is local to each core's rows.

The hot loop is DVE-only (4 ops per 2000-column tile) to minimize
cross-engine semaphore traffic; per-tile DVE busy ~8us vs ~11.4us of DMA
per tile, so the kernel rides the HBM roofline. The label one-hot is
  fix  = (iota == label - j*W) * delta        (one fused tensor_scalar)
  out  = cosr * 32 + fix                      (one scalar_tensor_tensor)
with delta = (phi - cos_l)*32 computed once per core from a 3-element
indirect-DMA gather of the label's subcenter block.
"""

import math
import sys

import numpy as np

if "/opt/trn_rl_repo" not in sys.path:
    sys.path.insert(0, "/opt/trn_rl_repo")

import concourse.bass as bass
import concourse.bacc as bacc
import concourse.mybir as mybir
from concourse.bass_utils import run_bass_kernel_spmd
from concourse.tile import TileContext

B = 1024
C = 20000          # out_features
K = 3              # subcenters
CK = C * K         # 60000
NCORES = 8
RB = B // NCORES   # 128 rows per core
W = 2000           # output columns per tile
NT = C // W        # tiles per core

SCALE = 32.0
MARGIN = 0.2
COS_M = math.cos(MARGIN)
SIN_M = math.sin(MARGIN)
TH = math.cos(math.pi - MARGIN)
MMM = 1.0 + math.cos(math.pi - MARGIN)

_CACHED_NC = None

import os
V_INT_IOTA = os.environ.get("V_INT_IOTA", "0") == "1"    # int32 iota + cast
V_ACT_MASK = os.environ.get("V_ACT_MASK", "0") == "1"    # ACT Abs/Relu mask instead of fused ts
V_ACT_SCALE = os.environ.get("V_ACT_SCALE", "0") == "1"  # ACT cos32 + tt_add instead of stt
V_NO_INPLACE = os.environ.get("V_NO_INPLACE", "0") == "1"  # separate cosr tile


def build():
    f32 = mybir.dt.float32
    i32 = mybir.dt.int32
    Alu = mybir.AluOpType
    Act = mybir.ActivationFunctionType

    nc = bacc.Bacc()
    cos_d = nc.declare_dram_parameter("cos", [RB, CK], f32, isOutput=False)
    gix_d = nc.declare_dram_parameter("gix", [RB, 1], i32, isOutput=False)
    labrel_d = nc.declare_dram_parameter("labrel", [RB, NT], f32, isOutput=False)
    out_d = nc.declare_dram_parameter("out", [RB, C], f32, isOutput=True)

    with TileContext(nc) as tc:
        with (
            tc.tile_pool(name="const", bufs=1) as cpool,
            tc.tile_pool(name="small", bufs=1) as spool,
            tc.tile_pool(name="inp", bufs=4) as ipool,
            tc.tile_pool(name="mid", bufs=3) as mpool,
        ):
            # iota 0..W-1 along the free dim, as f32 (exact < 2^24)
            iota_f = cpool.tile([RB, W], f32)
            if V_INT_IOTA:
                iota_i = cpool.tile([RB, W], i32)
                nc.gpsimd.iota(
                    iota_i[:], pattern=[[1, W]], base=0, channel_multiplier=0
                )
                nc.vector.tensor_copy(iota_f[:], iota_i[:])
            else:
                nc.gpsimd.iota(
                    iota_f[:], pattern=[[1, W]], base=0, channel_multiplier=0,
                    allow_small_or_imprecise_dtypes=True,
                )

            labrel_t = cpool.tile([RB, NT], f32)
            nc.sync.dma_start(out=labrel_t[:], in_=labrel_d[:])
            gix_t = cpool.tile([RB, 1], i32)
            nc.sync.dma_start(out=gix_t[:], in_=gix_d[:])

            # small per-partition constants (avoid const-AP registry)
            mmm_t = cpool.tile([RB, 1], f32)
            nc.gpsimd.memset(mmm_t[:], -MMM)
            th_t = cpool.tile([RB, 1], f32)
            nc.gpsimd.memset(th_t[:], TH)

            # Gather cosine[r, 3l : 3l+3] -> [RB, 3] via flat element offsets.
            g3 = spool.tile([RB, 3], f32)
            nc.gpsimd.indirect_dma_start(
                out=g3[:],
                out_offset=None,
                in_=cos_d[:].rearrange("p (n o) -> (p n) o", o=1),
                in_offset=bass.IndirectOffsetOnAxis(ap=gix_t[:, :1], axis=0),
            )
            cos_l = spool.tile([RB, 1], f32)
            nc.vector.tensor_reduce(
                out=cos_l[:], in_=g3[:], axis=mybir.AxisListType.X, op=Alu.max
            )

            # phi at the label column, then delta = (phi - cos_l) * SCALE.
            # All [RB,1] ops, mostly on ACT to keep DVE free.
            c2 = spool.tile([RB, 1], f32)
            nc.scalar.square(c2[:], cos_l[:])
            om = spool.tile([RB, 1], f32)
            nc.scalar.activation(om[:], c2[:], Act.Identity, bias=1.0, scale=-1.0)
            omc = spool.tile([RB, 1], f32)
            nc.scalar.activation(omc[:], om[:], Act.Relu)
            sine = spool.tile([RB, 1], f32)
            nc.scalar.sqrt(sine[:], omc[:])
            pa = spool.tile([RB, 1], f32)
            nc.scalar.mul(pa[:], cos_l[:], COS_M)
            pb = spool.tile([RB, 1], f32)
            nc.scalar.mul(pb[:], sine[:], -SIN_M)
            phi_b = spool.tile([RB, 1], f32)
            nc.vector.tensor_add(phi_b[:], pa[:], pb[:])
            cmm = spool.tile([RB, 1], f32)
            nc.vector.tensor_add(cmm[:], cos_l[:], mmm_t[:])
            mgt = spool.tile([RB, 1], f32)
            nc.vector.tensor_tensor(
                out=mgt[:], in0=cos_l[:], in1=th_t[:], op=Alu.is_gt
            )
            d1 = spool.tile([RB, 1], f32)
            nc.vector.tensor_sub(d1[:], phi_b[:], cmm[:])
            d2 = spool.tile([RB, 1], f32)
            nc.vector.tensor_mul(d2[:], d1[:], mgt[:])
            phi = spool.tile([RB, 1], f32)
            nc.vector.tensor_add(phi[:], d2[:], cmm[:])
            d3 = spool.tile([RB, 1], f32)
            nc.vector.tensor_sub(d3[:], phi[:], cos_l[:])
            delta = spool.tile([RB, 1], f32)
            nc.scalar.mul(delta[:], d3[:], SCALE)

            for j in range(NT):
                in3 = ipool.tile([RB, 3 * W], f32, tag="in3")
                nc.sync.dma_start(
                    out=in3[:], in_=cos_d[:, j * 3 * W : (j + 1) * 3 * W]
                )
                v = in3[:].rearrange("p (w k) -> p w k", k=3)
                t0 = mpool.tile([RB, W], f32, tag="t0")
                nc.vector.tensor_max(t0[:], v[:, :, 0], v[:, :, 1])
                if V_NO_INPLACE:
                    cosr = mpool.tile([RB, W], f32, tag="cosr")
                    nc.vector.tensor_max(cosr[:], t0[:], v[:, :, 2])
                else:
                    cosr = t0
                    nc.vector.tensor_max(t0[:], t0[:], v[:, :, 2])
                # fix = (iota == label - j*W) * delta
                fix = mpool.tile([RB, W], f32, tag="fix")
                if V_ACT_MASK:
                    absd = mpool.tile([RB, W], f32, tag="absd")
                    # |labrel - iota| == |iota - labrel|
                    nc.scalar.activation(
                        absd[:], iota_f[:], Act.Abs,
                        bias=labrel_t[:, j : j + 1], scale=-1.0,
                    )
                    nc.scalar.activation(
                        fix[:], absd[:], Act.Relu, bias=1.0, scale=-1.0
                    )
                    nc.scalar.activation(
                        fix[:], fix[:], Act.Copy, bias=0.0, scale=delta[:, :1]
                    )
                else:
                    nc.vector.tensor_scalar(
                        out=fix[:], in0=iota_f[:],
                        scalar1=labrel_t[:, j : j + 1], scalar2=delta[:, :1],
                        op0=Alu.is_equal, op1=Alu.mult,
                    )
                outt = mpool.tile([RB, W], f32, tag="outt")
                if V_ACT_SCALE:
                    nc.scalar.mul(outt[:], cosr[:], SCALE)
                    nc.vector.tensor_add(outt[:], outt[:], fix[:])
                else:
                    nc.vector.scalar_tensor_tensor(
                        out=outt[:], in0=cosr[:], scalar=SCALE, in1=fix[:],
                        op0=Alu.mult, op1=Alu.add,
                    )
                nc.sync.dma_start(out=out_d[:, j * W : (j + 1) * W], in_=outt[:])

    nc.finalize()
    return nc


def _make_in_maps(cosine: np.ndarray, label: np.ndarray):
    in_maps = []
    rows = np.arange(RB, dtype=np.int64)
    jw = (np.arange(NT, dtype=np.int64) * W)[None, :]  # [1, NT]
    for i in range(NCORES):
        rs = slice(i * RB, (i + 1) * RB)
        lab = np.asarray(label[rs], dtype=np.int64)
        gix = (rows * CK + 3 * lab).astype(np.int32).reshape(RB, 1)
        labrel = (lab[:, None] - jw).astype(np.float32)  # [RB, NT]
        in_maps.append(
            {
                "cos": np.ascontiguousarray(cosine[rs], dtype=np.float32),
                "gix": gix,
                "labrel": np.ascontiguousarray(labrel),
            }
        )
    return in_maps


def kernel(cosine: np.ndarray, label: np.ndarray) -> np.ndarray:
    global _CACHED_NC
    cosine = np.asarray(cosine)
    label = np.asarray(label)
    assert cosine.shape == (B, CK), cosine.shape
    assert label.shape == (B,), label.shape

    if _CACHED_NC is None:
        _CACHED_NC = build()
    nc = _CACHED_NC

    in_maps = _make_in_maps(cosine, label)
    res = run_bass_kernel_spmd(nc, in_maps, core_ids=list(range(NCORES)))
    out = np.concatenate([res.results[i]["out"] for i in range(NCORES)], axis=0)
    return out.astype(np.float32, copy=False)
